# revision 16
# baseline (speedup 1.0000x reference)
"""Trainium2 Bass kernel for additive-attention pooling (V5).

reference math:
    scores[b,t] = tanh(q[b]) @ vw_a + tanh(c[b,t]) @ vw_b
    attn        = softmax(where(mask<1, -1e10, scores), axis=t)
    out[b,e]    = sum_t attn[b,t] * c[b,t,e]

Softmax is shift-invariant and the query term is constant over t, so the
output depends only on `context`, `mask`, and `v_w[E:]`.

Measured per-core engine economics (768-elem slice granularity):
  - Any 2-tensor-stream DVE op (affine_mul_reduce / scalar_tensor_tensor
    / tensor_tensor) costs ~0.95-1.2us regardless of dtype; no 2x mode.
  - A PE matmul pair per slice (m=1 lhsT) costs ~1.3-1.5us and is stuck
    at the cold-isolated rate (no FWL below 128 weight columns).
  - Engines slow each other ~15-20% when co-streaming from SBUF, so
    DVE->PE offload is nearly a wash; minimizing total slices wins.
  - tanh on ACT: 1 elem/lane/cycle at 1.2 GHz, dtype-independent.

Structure:
1. Host-side row compaction: masked rows (~50%, known host-side) are
   dropped; each batch row packs to T2=2176 rows in tiles of
   [128x(6,6,5)] rows (seeded inputs: max k=2100; any larger mask lazily
   rebuilds a bigger program).  Pad rows are zero: tanh(0)=0 -> score 0
   -> p=1, adding 0 to the numerator and +1 per pad to the denominator,
   cancelled exactly by seeding the den accumulator with host-computed
   -npad per partition.
2. bf16 packed context: 6.7 MB HBM traffic, fully contiguous
   per-partition DMA lines.
3. Per tile: ACT tanh (bf16 out) -> DVE affine_mul_reduce per j-slice
   (score dot, bf16 streams to cut SBUF contention) -> ACT exp (bf16
   weights + free accum_out denominator)
   -> weighted sum: PE matmul pairs (lhsT = exp weights) for most
   j-slices, DVE scalar_tensor_tensor into an f32 accumulator for the
   last tile's slices (DVE is score-idle by then).  PSUM accumulates
   across tiles; a final constant-ones matmul pair folds the DVE
   accumulator in, and a 1-column matmul reduces the denominator.
4. ACT warmup activation at t=0 so the tanh/exp table set loads under
   the first DMA; all context DMAs are front-loaded (SBUF holds every
   tile) so the queue drains by ~25us and the DVE's back half runs
   without DMA co-streaming — this also makes run times deterministic.

Sharding: pure data parallel, batch 16 -> 2 per core on 8 cores; w2
replicated.  No collectives.
"""

import sys

for _p in ("/opt/trn_rl_repo", "/root/.axon_site/_ro/trn_rl_repo"):
    if _p not in sys.path:
        sys.path.append(_p)

import numpy as np
from ml_dtypes import bfloat16

B, T, E = 16, 4096, 768
NCORES = 8
BPC = B // NCORES   # batch rows per core
P = 128             # partitions per tile
T2_DEFAULT = 2176   # packed row count (multiple of 128, >= max unmasked)
EB = E + 1          # output row: 768 numerator + 1 denominator

_cache = {}


def _tile_plan(T2):
    """Split T2 rows into tiles of [P x Jt] rows, J=6 then a remainder
    tile.  Returns [(row_start, Jt), ...]."""
    assert T2 % P == 0
    tiles = []
    start = 0
    while T2 - start >= 6 * P:
        rem = (T2 - start) // P
        # avoid leaving a tiny remainder tile (J>=2 keeps DMA lines fat)
        j = 6 if rem >= 8 or rem == 6 else rem - 2
        tiles.append((start, j))
        start += j * P
    if start < T2:
        tiles.append((start, (T2 - start) // P))
    return tiles


def _build_program(T2):
    import concourse.tile as tile
    from concourse import bacc, mybir

    f32 = mybir.dt.float32
    bf16 = mybir.dt.bfloat16
    AF = mybir.ActivationFunctionType
    ALU = mybir.AluOpType
    tiles = _tile_plan(T2)
    NT = len(tiles)

    nc = bacc.Bacc(
        "TRN2",
        target_bir_lowering=False,
        debug=False,
        enable_asserts=False,
        num_devices=NCORES,
    )
    ctx_d = nc.dram_tensor("ctxp", [BPC, T2, E], bf16, kind="ExternalInput")
    w2_d = nc.dram_tensor("w2rep", [P, E], bf16, kind="ExternalInput")
    negn_d = nc.dram_tensor("negnpad", [BPC, P, 1], f32, kind="ExternalInput")
    out_d = nc.dram_tensor("out", [BPC, EB], f32, kind="ExternalOutput")

    # tiles interleaved across the two batch rows so each batch's final
    # matmul+store overlaps the other batch's compute
    seq = [(b, t) for t in range(NT) for b in range(BPC)]
    NSEQ = len(seq)
    last_of_batch = {b: max(i for i, (bb, _) in enumerate(seq) if bb == b)
                     for b in range(BPC)}

    def dve_js(i):
        """Prereduce j-slices handled by DVE stt (rest go to PE).  Put
        them on the last tiles, where the DVE's score pass has wound
        down; early tiles go all-PE."""
        b, t = seq[i]
        jt = tiles[t][1]
        if t == NT - 1:
            return tuple(range(jt - 3, jt))
        return ()

    with tile.TileContext(nc) as tc:
        with (
            tc.tile_pool(name="const", bufs=1) as const_pool,
            tc.tile_pool(name="cin", bufs=6) as c_pool,
            tc.tile_pool(name="tanh", bufs=3) as t_pool,
            tc.tile_pool(name="small", bufs=12) as s_pool,
            tc.tile_pool(name="accs", bufs=BPC) as a_pool,
            tc.tile_pool(name="outp", bufs=BPC) as o_pool,
            tc.tile_pool(name="paccum", bufs=BPC, space="PSUM") as pa_pool,
        ):
            cs = {}

            def dma_tile(i):
                b, t = seq[i]
                start, jt = tiles[t]
                c = c_pool.tile([P, 6 * E], bf16)
                nc.sync.dma_start(
                    c[:, 0:jt * E].rearrange("p (j e) -> p j e", j=jt),
                    ctx_d[b, start:start + jt * P, :].rearrange(
                        "(p j) e -> p j e", j=jt
                    ),
                )
                cs[i] = c

            # first context tile ahead of everything: compute ramps earliest
            dma_tile(0)

            w2_t = const_pool.tile([P, E], bf16)
            nc.sync.dma_start(w2_t[:], w2_d[:])
            ones = const_pool.tile([P, 1], bf16)
            nc.gpsimd.memset(ones[:], 1.0)
            # dummy activation so the tanh/exp table set loads during the
            # first context DMA instead of serializing in front of tanh(0)
            warm = const_pool.tile([P, 1], f32)
            nc.gpsimd.memset(warm[:], 0.0)
            warm2 = const_pool.tile([P, 1], f32)
            nc.scalar.activation(warm2[:], warm[:], AF.Tanh)

            # per-batch state: f32 accumulator for the DVE-side partial
            # [sum p*c | sum p], PSUM accumulator for the PE-side groups,
            # and first-write flags per PSUM region
            accs, psums, started = [], [], []
            for b in range(BPC):
                acc = a_pool.tile([P, EB], f32)
                nc.gpsimd.memset(acc[:, 0:E], 0.0)
                nc.sync.dma_start(acc[:, E:EB], negn_d[b])
                accs.append(acc)
                ps = pa_pool.tile([1, EB], f32, name=f"ps{b}")
                psums.append(ps)
                started.append([False, False])

            # front-load the remaining context DMAs: the queue drains by
            # ~25us, so the DVE's back half runs without DMA co-streaming
            for _i in range(1, NSEQ):
                dma_tile(_i)

            ths = {}

            def tanh_tile(i):
                b, t = seq[i]
                jt = tiles[t][1]
                c = cs[i]
                th = t_pool.tile([P, 6 * E], bf16)
                nc.scalar.activation(th[:, 0:jt * E], c[:, 0:jt * E], AF.Tanh)
                ths[i] = th

            def score_reduce(i):
                b, t = seq[i]
                jt = tiles[t][1]
                th = ths.pop(i)
                c = cs.pop(i)
                acc = accs[b]
                ps = psums[b]
                s2 = s_pool.tile([P, 6], f32)
                for j in range(jt):
                    sl = slice(j * E, (j + 1) * E)
                    nc.vector.affine_mul_reduce(
                        th[:, sl], s2[:, j:j + 1], th[:, sl], w2_t[:],
                        1.0, 0.0,
                    )
                p2b = s_pool.tile([P, 6], bf16)
                denj = s_pool.tile([P, 1], f32)
                nc.scalar.activation(p2b[:, 0:jt], s2[:, 0:jt], AF.Exp,
                                     accum_out=denj[:])
                dj = dve_js(i)
                for j in dj:
                    sl = slice(j * E, (j + 1) * E)
                    nc.vector.scalar_tensor_tensor(
                        acc[:, 0:E], c[:, sl], p2b[:, j:j + 1], acc[:, 0:E],
                        op0=ALU.mult, op1=ALU.add,
                    )
                nc.vector.tensor_add(acc[:, E:EB], acc[:, E:EB], denj[:])
                for j in range(jt):
                    if j in dj:
                        continue
                    nc.tensor.matmul(
                        ps[:, 0:512], lhsT=p2b[:, j:j + 1],
                        rhs=c[:, j * E:j * E + 512],
                        start=not started[b][0], stop=False,
                    )
                    started[b][0] = True
                    nc.tensor.matmul(
                        ps[:, 512:E], lhsT=p2b[:, j:j + 1],
                        rhs=c[:, j * E + 512:(j + 1) * E],
                        start=not started[b][1], stop=False,
                    )
                    started[b][1] = True

            def batch_final(b):
                acc = accs[b]
                ps = psums[b]
                # zero-cost truncated-bf16 view of the f32 accumulator
                accv = acc[:].bitcast(bf16).rearrange(
                    "p (n two) -> p n two", two=2
                )[:, :, 1]
                nc.tensor.matmul(ps[:, 0:512], lhsT=ones[:],
                                 rhs=accv[:, 0:512],
                                 start=not started[b][0], stop=True)
                nc.tensor.matmul(ps[:, 512:E], lhsT=ones[:],
                                 rhs=accv[:, 512:E],
                                 start=not started[b][1], stop=True)
                nc.tensor.matmul(ps[:, E:EB], lhsT=ones[:],
                                 rhs=accv[:, E:EB], start=True, stop=True)
                out_sb = o_pool.tile([1, EB], f32)
                nc.scalar.activation(out_sb[:], ps[:], AF.Copy)
                nc.sync.dma_start(out_d[b:b + 1, :], out_sb[:])

            # software-pipelined emission: tanh runs 2 tiles ahead of the
            # score/reduce stage so the ACT FIFO never stalls behind an
            # exp that waits on the DVE
            tanh_tile(0)
            tanh_tile(1)
            for i in range(NSEQ):
                if i + 2 < NSEQ:
                    tanh_tile(i + 2)
                score_reduce(i)
                b, _ = seq[i]
                if i == last_of_batch[b]:
                    batch_final(b)

    nc.compile()
    return nc


def _get_program(T2=T2_DEFAULT):
    key = ("nc", T2)
    if key not in _cache:
        _cache[key] = _build_program(T2)
    return _cache[key]


def _prepare(query, context, mask, v_w):
    """Host-side pack: compact unmasked rows, pad to T2, bf16-cast.
    Returns (T2, in_maps, k) where k[b] = unmasked row count."""
    mask = np.asarray(mask)
    context = np.asarray(context, dtype=np.float32)
    v_w = np.asarray(v_w, dtype=np.float32)

    k = (mask != 0).sum(axis=1).astype(np.int64)
    T2 = T2_DEFAULT
    if k.max() > T2:
        T2 = int(-(-k.max() // P) * P)  # ceil to 128 rows

    packed = np.zeros((B, T2, E), dtype=bfloat16)
    for b in range(B):
        idx = np.flatnonzero(mask[b])
        packed[b, :k[b]] = context[b, idx].astype(bfloat16)

    # per-partition pad counts for the den seed: within tile (start, jt),
    # row r maps to partition (r - start) // jt
    r_part = np.empty(T2, dtype=np.int64)
    for start, jt in _tile_plan(T2):
        rr = np.arange(start, start + jt * P)
        r_part[rr] = (rr - start) // jt
    negn = np.zeros((B, P, 1), dtype=np.float32)
    for b in range(B):
        pads = r_part[k[b]:]
        np.subtract.at(negn[b, :, 0], pads, 1.0)

    w2_rep = np.ascontiguousarray(
        np.broadcast_to(v_w[E:], (P, E)).astype(bfloat16))

    in_maps = [
        {
            "ctxp": np.ascontiguousarray(packed[i * BPC:(i + 1) * BPC]),
            "w2rep": w2_rep,
            "negnpad": np.ascontiguousarray(negn[i * BPC:(i + 1) * BPC]),
        }
        for i in range(NCORES)
    ]
    return T2, in_maps, k


def kernel(query, context, mask, v_w):
    import time
    from concourse.bass_utils import run_bass_kernel_spmd

    T2, in_maps, _ = _prepare(query, context, mask, v_w)
    nc = _get_program(T2)
    last_err = None
    for attempt in range(3):
        try:
            res = run_bass_kernel_spmd(nc, in_maps, list(range(NCORES)))
            raw = np.concatenate(
                [res.results[i]["out"] for i in range(NCORES)], axis=0
            )
            return (raw[:, :E] / raw[:, E:EB]).astype(np.float32)
        except Exception as e:  # transient axon/device hiccups
            last_err = e
            time.sleep(5)
    raise last_err
